# revision 37
# baseline (speedup 1.0000x reference)
"""Trainium2 Bass kernel for nn_Attention_36404142801494.

Fused causal self-attention (q=k=v=Wq(x)) + output projection, sharded over
8 NeuronCores: data-parallel on batch (B=2 -> 2 groups of 4 cores), tensor-
parallel on heads (8 heads -> 2 heads/core) with a column-split Wq and a
row-split Wo. Each core returns a partial [S, HID] output; the host sums the
4 partials per batch and adds the Wo bias while unsharding.

Layout strategy on device (per core):
  - qT [d, s] layout (d on partitions) so QK^T tiles come out as
    scoresT [t_keys=128, s_queries=512] and feed the AV matmul directly.
  - scores for one (head, 512-query block) are built 3 key-chunks at a time
    into a 3-bank PSUM group [128, 1536] (double buffered), exponentiated by
    one ACT pass per group (scale=1/8 folded in, bf16 out), causal-masked
    per diagonal segment via precomputed 0/1 mask multiplies.
  - V tiles [t, d] come from PE transposes of qT; 32 extra "ones" columns
    make the AV matmul emit softmax denominators (rows 64..95 of the AV
    accumulator) for free.
  - Normalization: denominator row -> DRAM bounce -> partition-broadcast
    DMA -> reciprocal -> one multiply while copying the AV result out.
  - QK / projections run as float32r (full-rate fp32 PE mode, ~1.6e-4 rel
    err); exp'd scores and V run in bf16 (post-softmax values, error is
    averaged out by the AV reduction).

Everything is hardcoded for B=2, S=2048, HID=512, NH=8, HD=64.
"""

import sys

sys.path.insert(0, "/opt/trn_rl_repo")

import numpy as np

import concourse.bass as bass
import concourse.bacc as bacc
import concourse.tile as tile
import concourse.mybir as mybir
from concourse.bass_utils import run_bass_kernel_spmd
from concourse.masks import make_identity

f32 = mybir.dt.float32
f32r = mybir.dt.float32r
bf16 = mybir.dt.bfloat16
EXPT_DT = bf16  # dtype of exp'd scores + V (AV matmul operands)

B, S, HID = 2, 2048, 512
NH, HD = 8, 64
N_CORES = 8
SB = 512           # query-block width (one PSUM bank of fp32)
NSB = S // SB      # 4 query blocks
NCH = S // 128     # 16 key chunks
GRP = 2            # key chunks per PSUM scores group (2 banks, double buffered)
SCALE = 1.0 / np.sqrt(HD)

Exp = mybir.ActivationFunctionType.Exp


def build_nc():
    """Build the (identical-on-every-core) Bass program."""
    nc = bacc.Bacc(None, target_bir_lowering=False)

    xT = nc.dram_tensor("xT", [HID, S], f32, kind="ExternalInput")
    WqT = nc.dram_tensor("WqT", [HID, 128], f32, kind="ExternalInput")
    Wqb = nc.dram_tensor("Wqb", [128, 1], f32, kind="ExternalInput")
    WoT = nc.dram_tensor("WoT", [128, HID], f32, kind="ExternalInput")
    dmask = nc.dram_tensor("dmask", [128, 2048], f32, kind="ExternalInput")
    out_part = nc.dram_tensor("out_part", [S, HID], f32, kind="ExternalOutput")

    with tile.TileContext(nc) as tc:
        with (
            tc.tile_pool(name="singles", bufs=1) as singles,
            tc.tile_pool(name="qtp", bufs=1) as qtp,
            tc.tile_pool(name="etp", bufs=12) as etp,
            tc.tile_pool(name="vp", bufs=1) as vp,
            tc.tile_pool(name="aop", bufs=1) as aop,
            tc.tile_pool(name="np_", bufs=4) as np_,
            tc.tile_pool(name="outp", bufs=4) as outp,
            tc.tile_pool(name="drp", bufs=2, space="DRAM") as drp,
        ):
            # ---- load constants / inputs (qproj-critical ones first) ----
            wq = singles.tile([128, 4, 128], f32r, tag="wq")
            for i in range(4):
                nc.sync.dma_start(
                    out=wq[:, i, :], in_=WqT[128 * i : 128 * (i + 1), :].bitcast(f32r)
                )
            wqb = singles.tile([128, 1], f32, tag="wqb")
            nc.sync.dma_start(out=wqb, in_=Wqb[:, :])

            # xT loaded per 512-column block so qproj can start early
            xs = [singles.tile([128, S], f32r, name=f"xt{i}", tag=f"xt{i}") for i in range(4)]
            for sb in range(NSB):
                s0 = sb * SB
                for i in range(4):
                    nc.sync.dma_start(
                        out=xs[i][:, s0 : s0 + SB],
                        in_=xT[128 * i : 128 * (i + 1), s0 : s0 + SB].bitcast(f32r),
                    )

            # non-critical loads go through the gpsimd DMA queue
            wo = singles.tile([64, 2 * HID], f32r, tag="wo")
            nc.gpsimd.dma_start(out=wo[:, 0:HID], in_=WoT[0:64, :].bitcast(f32r))
            nc.gpsimd.dma_start(out=wo[:, HID : 2 * HID], in_=WoT[64:128, :].bitcast(f32r))
            dm = singles.tile([128, 2048], EXPT_DT, tag="dm")
            dmf = singles.tile([128, 2048], f32, tag="dmf")
            nc.gpsimd.dma_start(out=dmf, in_=dmask[:, :])
            nc.gpsimd.tensor_copy(dm, dmf)

            # preload the exp ACT table while DMAs stream in
            preld = singles.tile([32, 32], f32, tag="preld")
            nc.vector.memset(preld, 0.0)
            nc.scalar.activation(out=preld, in_=preld, func=Exp, scale=1.0)

            identf = singles.tile([128, 64], f32, tag="identf")
            make_identity(nc, identf[0:64, :])
            nc.gpsimd.memset(identf[64:128, :], 0.0)
            nc.gpsimd.affine_select(
                out=identf[64:128, :], in_=identf[64:128, :],
                compare_op=mybir.AluOpType.not_equal,
                fill=1.0, base=0, pattern=[[-1, 64]], channel_multiplier=1,
            )
            ident = singles.tile([128, 64], f32r, tag="ident")
            nc.vector.tensor_copy(ident, identf)
            onesf = singles.tile([128, 32], f32, tag="onesf")
            nc.vector.memset(onesf, 1.0)

            qT = qtp.tile([128, S], f32r, tag="qT")
            v_sb = [vp.tile([128, NCH, 96], EXPT_DT, name=f"v{h}", tag=f"v{h}") for h in range(2)]
            ao = [aop.tile([64, S], f32r, name=f"ao{h}", tag=f"ao{h}") for h in range(2)]

            # ---- phase 1: q projection (qT = Wq @ x^T + b) and V prep ----


            # ---- main pipeline ----
            def vprep(vps, h, tq):
                hp = 64 * h
                vt = vps.tile([128, 4, 64], f32r, tag="ps1", name="vt", bufs=1)
                for j in range(4):
                    t0 = 128 * (4 * tq + j)
                    nc.tensor.transpose(
                        vt[:, j, :], qT[hp : hp + 64, t0 : t0 + 128],
                        ident[hp : hp + 64, :],
                    )
                nc.vector.tensor_copy(v_sb[h][:, 4 * tq : 4 * tq + 4, 0:64], vt)
                for j in range(4):
                    nc.gpsimd.tensor_copy(v_sb[h][:, 4 * tq + j, 64:96], onesf)

            def attention(qkps, avps, h, sb, tail=False):
                hp = 64 * h
                s0 = sb * SB
                nch = 4 * (sb + 1)
                groups = [
                    list(range(g0, min(g0 + GRP, nch))) for g0 in range(0, nch, GRP)
                ]
                av = avps.tile([96, SB], f32, tag="av", name="av", bufs=3)
                ets = []
                for chunks in groups:
                    gw = len(chunks)
                    qk = qkps.tile([128, GRP * SB], f32, tag="qk", name="qk")
                    for k, ci in enumerate(chunks):
                        t0 = 128 * ci
                        nc.tensor.matmul(
                            qk[:, SB * k : SB * (k + 1)],
                            lhsT=qT[hp : hp + 64, t0 : t0 + 128],
                            rhs=qT[hp : hp + 64, s0 : s0 + SB],
                            start=True, stop=True,
                        )
                    et = etp.tile([128, GRP * SB], EXPT_DT, tag="et", name="et")
                    nc.scalar.activation(
                        out=et[:, 0 : gw * SB], in_=qk[:, 0 : gw * SB],
                        func=Exp, scale=SCALE,
                    )
                    for k, ci in enumerate(chunks):
                        d = ci - 4 * sb
                        if d >= 0:
                            nc.vector.tensor_mul(
                                et[:, SB * k : SB * (k + 1)],
                                et[:, SB * k : SB * (k + 1)],
                                dm[:, SB * d : SB * (d + 1)],
                            )
                    ets.append((chunks, et))
                for chunks, et in ets:
                    for k, ci in enumerate(chunks):
                        nc.tensor.matmul(
                            av,
                            lhsT=v_sb[h][:, ci, :],
                            rhs=et[:, SB * k : SB * (k + 1)],
                            start=(ci == 0), stop=(ci == nch - 1),
                        )
                # normalization: denom rows 64..95 -> recip -> scale
                den = np_.tile([96, SB], f32, tag="den", name="den")
                if tail:
                    nc.scalar.copy(den, av)
                else:
                    nc.vector.tensor_copy(den, av)
                dr = drp.tile([1, SB], f32, name="dr")
                nc.gpsimd.dma_start(out=dr[:, :], in_=den[64:65, :])
                drap = dr[:, :]
                bcast = bass.AP(
                    tensor=drap.tensor, offset=drap.offset,
                    ap=[[0, 64]] + list(drap.ap)[1:],
                )
                bcr = np_.tile([64, SB], f32, tag="bcr", name="bcr")
                nc.gpsimd.dma_start(out=bcr, in_=bcast)
                bc = np_.tile([64, SB], f32, tag="bc", name="bc")
                nc.vector.reciprocal(bc, bcr)
                nc.vector.tensor_mul(ao[h][:, s0 : s0 + SB], den[0:64, :], bc)

            def wo_block(wops, sb):
                for sc in range(4 * sb, 4 * sb + 4):
                    c0 = 128 * sc
                    wp = wops.tile([128, HID], f32, tag="ps1", name="wp", bufs=1)
                    nc.tensor.matmul(
                        wp, lhsT=ao[0][:, c0 : c0 + 128], rhs=wo[:, 0:HID],
                        start=True, stop=False,
                    )
                    nc.tensor.matmul(
                        wp, lhsT=ao[1][:, c0 : c0 + 128], rhs=wo[:, HID : 2 * HID],
                        start=False, stop=True,
                    )
                    ob = outp.tile([128, HID], f32, tag="ob", name="ob")
                    nc.vector.tensor_copy(ob, wp)
                    nc.sync.dma_start(out=out_part[c0 : c0 + 128, :], in_=ob)

            with (
                tc.tile_pool(name="qpps", bufs=2, space="PSUM") as qpps,
                tc.tile_pool(name="qkps", bufs=2, space="PSUM") as qkps,
                tc.tile_pool(name="avps", bufs=2, space="PSUM") as avps,
            ):
                # qpps doubles as the vprep transpose pool and the Wo pool:
                # qproj ends as vprep starts, Wo comes later still.
                for sb in range(NSB):
                    s0 = sb * SB
                    qp = qpps.tile([128, SB], f32, tag="ps1", name="qp", bufs=1)
                    for i in range(4):
                        nc.tensor.matmul(
                            qp, lhsT=wq[:, i, :], rhs=xs[i][:, s0 : s0 + SB],
                            start=(i == 0), stop=(i == 3),
                        )
                    nc.vector.tensor_scalar_add(qT[:, s0 : s0 + SB], qp, wqb)
                    vprep(qpps, 0, sb)
                    vprep(qpps, 1, sb)
                    attention(qkps, avps, 0, sb)
                for sb in (3, 2, 1, 0):
                    attention(qkps, avps, 1, sb, tail=(sb <= 1))
                    wo_block(qpps, sb)

    nc.finalize()
    return nc


def _dmask():
    """[128, 2048] mask, segment d in {0..3}: keep (t + 128*d) <= j."""
    t = np.arange(128)[:, None]
    j = np.arange(512)[None, :]
    segs = [(t + 128 * k <= j).astype(np.float32) for k in range(4)]
    return np.concatenate(segs, axis=1)


_NC_CACHE = None


def _get_nc():
    global _NC_CACHE
    if _NC_CACHE is None:
        _NC_CACHE = build_nc()
    return _NC_CACHE


def make_in_maps(x, Wq_w, Wq_b, Wo_w):
    x = np.asarray(x, dtype=np.float32)
    Wq_w = np.asarray(Wq_w, dtype=np.float32)
    Wq_b = np.asarray(Wq_b, dtype=np.float32)
    Wo_w = np.asarray(Wo_w, dtype=np.float32)
    dmask = _dmask()
    in_maps = []
    for c in range(N_CORES):
        b, hp = divmod(c, 4)
        dq = slice(128 * hp, 128 * (hp + 1))
        in_maps.append({
            "xT": np.ascontiguousarray(x[b].T),
            "WqT": np.ascontiguousarray(Wq_w[dq, :].T),
            "Wqb": np.ascontiguousarray(Wq_b[dq].reshape(128, 1)),
            "WoT": np.ascontiguousarray(Wo_w[:, dq].T),
            "dmask": dmask,
        })
    return in_maps


def kernel(x, mask, Wq_w, Wq_b, Wo_w, Wo_b, **_):
    nc = _get_nc()
    in_maps = make_in_maps(x, Wq_w, Wq_b, Wo_w)
    res = run_bass_kernel_spmd(nc, in_maps, core_ids=list(range(N_CORES)))
    Wo_b = np.asarray(Wo_b, dtype=np.float32)
    out = np.empty((B, S, HID), dtype=np.float32)
    for b in range(B):
        acc = res.results[4 * b]["out_part"].astype(np.float32)
        for c in range(4 * b + 1, 4 * b + 4):
            acc = acc + res.results[c]["out_part"]
        out[b] = acc + Wo_b[None, :]
    return out


# revision 38
# speedup vs baseline: 1.0597x; 1.0597x over previous
"""Trainium2 Bass kernel for nn_Attention_36404142801494.

Fused causal self-attention (q=k=v=Wq(x)) + output projection, sharded over
8 NeuronCores: data-parallel on batch (B=2 -> 2 groups of 4 cores), tensor-
parallel on heads (8 heads -> 2 heads/core) with a column-split Wq and a
row-split Wo. Each core returns a partial [S, HID] output; the host sums the
4 partials per batch and adds the Wo bias while unsharding.

Layout strategy on device (per core):
  - qT [d, s] layout (d on partitions) so QK^T tiles come out as
    scoresT [t_keys=128, s_queries=512] and feed the AV matmul directly.
  - scores for one (head, 512-query block) are built 3 key-chunks at a time
    into a 3-bank PSUM group [128, 1536] (double buffered), exponentiated by
    one ACT pass per group (scale=1/8 folded in, bf16 out), causal-masked
    per diagonal segment via precomputed 0/1 mask multiplies.
  - V tiles [t, d] come from PE transposes of qT; 32 extra "ones" columns
    make the AV matmul emit softmax denominators (rows 64..95 of the AV
    accumulator) for free.
  - Normalization: denominator row -> DRAM bounce -> partition-broadcast
    DMA -> reciprocal -> one multiply while copying the AV result out.
  - QK / projections run as float32r (full-rate fp32 PE mode, ~1.6e-4 rel
    err); exp'd scores and V run in bf16 (post-softmax values, error is
    averaged out by the AV reduction).

Everything is hardcoded for B=2, S=2048, HID=512, NH=8, HD=64.
"""

import sys

sys.path.insert(0, "/opt/trn_rl_repo")

import numpy as np

import concourse.bass as bass
import concourse.bacc as bacc
import concourse.tile as tile
import concourse.mybir as mybir
from concourse.bass_utils import run_bass_kernel_spmd
from concourse.masks import make_identity

f32 = mybir.dt.float32
f32r = mybir.dt.float32r
bf16 = mybir.dt.bfloat16
EXPT_DT = bf16  # dtype of exp'd scores + V (AV matmul operands)

B, S, HID = 2, 2048, 512
NH, HD = 8, 64
N_CORES = 8
SB = 512           # query-block width (one PSUM bank of fp32)
NSB = S // SB      # 4 query blocks
NCH = S // 128     # 16 key chunks
GRP = 2            # key chunks per PSUM scores group (2 banks, double buffered)
SCALE = 1.0 / np.sqrt(HD)

Exp = mybir.ActivationFunctionType.Exp


def build_nc():
    """Build the (identical-on-every-core) Bass program."""
    nc = bacc.Bacc(None, target_bir_lowering=False)

    xT = nc.dram_tensor("xT", [HID, S], f32, kind="ExternalInput")
    WqT = nc.dram_tensor("WqT", [HID, 128], f32, kind="ExternalInput")
    Wqb = nc.dram_tensor("Wqb", [128, 1], f32, kind="ExternalInput")
    WoT = nc.dram_tensor("WoT", [128, HID], f32, kind="ExternalInput")
    dmask = nc.dram_tensor("dmask", [128, 2048], f32, kind="ExternalInput")
    out_part = nc.dram_tensor("out_part", [S, HID], f32, kind="ExternalOutput")

    with tile.TileContext(nc) as tc:
        with (
            tc.tile_pool(name="singles", bufs=1) as singles,
            tc.tile_pool(name="qtp", bufs=1) as qtp,
            tc.tile_pool(name="etp", bufs=12) as etp,
            tc.tile_pool(name="vp", bufs=1) as vp,
            tc.tile_pool(name="aop", bufs=1) as aop,
            tc.tile_pool(name="np_", bufs=4) as np_,
            tc.tile_pool(name="outp", bufs=4) as outp,
            tc.tile_pool(name="drp", bufs=2, space="DRAM") as drp,
        ):
            # ---- load constants / inputs (qproj-critical ones first) ----
            wq = singles.tile([128, 4, 128], f32r, tag="wq")
            for i in range(4):
                nc.sync.dma_start(
                    out=wq[:, i, :], in_=WqT[128 * i : 128 * (i + 1), :].bitcast(f32r)
                )
            wqb = singles.tile([128, 1], f32, tag="wqb")
            nc.sync.dma_start(out=wqb, in_=Wqb[:, :])

            # xT loaded per 512-column block so qproj can start early
            xs = [singles.tile([128, S], f32r, name=f"xt{i}", tag=f"xt{i}") for i in range(4)]
            for sb in range(NSB):
                s0 = sb * SB
                for i in range(4):
                    nc.sync.dma_start(
                        out=xs[i][:, s0 : s0 + SB],
                        in_=xT[128 * i : 128 * (i + 1), s0 : s0 + SB].bitcast(f32r),
                    )

            # non-critical loads go through the gpsimd DMA queue
            wo = singles.tile([64, 2 * HID], f32r, tag="wo")
            nc.gpsimd.dma_start(out=wo[:, 0:HID], in_=WoT[0:64, :].bitcast(f32r))
            nc.gpsimd.dma_start(out=wo[:, HID : 2 * HID], in_=WoT[64:128, :].bitcast(f32r))
            dm = singles.tile([128, 2048], EXPT_DT, tag="dm")
            dmf = singles.tile([128, 2048], f32, tag="dmf")
            nc.gpsimd.dma_start(out=dmf, in_=dmask[:, :])
            nc.gpsimd.tensor_copy(dm, dmf)

            # preload the exp ACT table while DMAs stream in
            preld = singles.tile([32, 32], f32, tag="preld")
            nc.vector.memset(preld, 0.0)
            nc.scalar.activation(out=preld, in_=preld, func=Exp, scale=1.0)

            identf = singles.tile([128, 64], f32, tag="identf")
            make_identity(nc, identf[0:64, :])
            nc.gpsimd.memset(identf[64:128, :], 0.0)
            nc.gpsimd.affine_select(
                out=identf[64:128, :], in_=identf[64:128, :],
                compare_op=mybir.AluOpType.not_equal,
                fill=1.0, base=0, pattern=[[-1, 64]], channel_multiplier=1,
            )
            ident = singles.tile([128, 64], f32r, tag="ident")
            nc.vector.tensor_copy(ident, identf)
            onesf = singles.tile([128, 32], f32, tag="onesf")
            nc.vector.memset(onesf, 1.0)

            qT = qtp.tile([128, S], f32r, tag="qT")
            v_sb = [vp.tile([128, NCH, 96], EXPT_DT, name=f"v{h}", tag=f"v{h}") for h in range(2)]
            ao = [aop.tile([64, S], f32r, name=f"ao{h}", tag=f"ao{h}") for h in range(2)]

            # ---- phase 1: q projection (qT = Wq @ x^T + b) and V prep ----


            # ---- main pipeline ----
            def vprep(vps, h, tq):
                hp = 64 * h
                vt = vps.tile([128, 4, 64], f32r, tag="ps1", name="vt")
                for j in range(4):
                    t0 = 128 * (4 * tq + j)
                    nc.tensor.transpose(
                        vt[:, j, :], qT[hp : hp + 64, t0 : t0 + 128],
                        ident[hp : hp + 64, :],
                    )
                nc.vector.tensor_copy(v_sb[h][:, 4 * tq : 4 * tq + 4, 0:64], vt)
                for j in range(4):
                    nc.gpsimd.tensor_copy(v_sb[h][:, 4 * tq + j, 64:96], onesf)

            def attention(qkps, avps, h, sb, tail=False):
                hp = 64 * h
                s0 = sb * SB
                nch = 4 * (sb + 1)
                groups = [
                    list(range(g0, min(g0 + GRP, nch))) for g0 in range(0, nch, GRP)
                ]
                av = avps.tile([96, SB], f32, tag="av", name="av")
                ets = []
                for chunks in groups:
                    gw = len(chunks)
                    qk = qkps.tile([128, GRP * SB], f32, tag="qk", name="qk")
                    for k, ci in enumerate(chunks):
                        t0 = 128 * ci
                        nc.tensor.matmul(
                            qk[:, SB * k : SB * (k + 1)],
                            lhsT=qT[hp : hp + 64, t0 : t0 + 128],
                            rhs=qT[hp : hp + 64, s0 : s0 + SB],
                            start=True, stop=True,
                        )
                    et = etp.tile([128, GRP * SB], EXPT_DT, tag="et", name="et")
                    nc.scalar.activation(
                        out=et[:, 0 : gw * SB], in_=qk[:, 0 : gw * SB],
                        func=Exp, scale=SCALE,
                    )
                    for k, ci in enumerate(chunks):
                        d = ci - 4 * sb
                        if d >= 0:
                            nc.vector.tensor_mul(
                                et[:, SB * k : SB * (k + 1)],
                                et[:, SB * k : SB * (k + 1)],
                                dm[:, SB * d : SB * (d + 1)],
                            )
                    ets.append((chunks, et))
                for chunks, et in ets:
                    for k, ci in enumerate(chunks):
                        nc.tensor.matmul(
                            av,
                            lhsT=v_sb[h][:, ci, :],
                            rhs=et[:, SB * k : SB * (k + 1)],
                            start=(ci == 0), stop=(ci == nch - 1),
                        )
                # normalization: denom rows 64..95 -> recip -> scale
                den = np_.tile([96, SB], f32, tag="den", name="den")
                if tail:
                    nc.scalar.copy(den, av)
                else:
                    nc.vector.tensor_copy(den, av)
                dr = drp.tile([1, SB], f32, name="dr")
                nc.gpsimd.dma_start(out=dr[:, :], in_=den[64:65, :])
                drap = dr[:, :]
                bcast = bass.AP(
                    tensor=drap.tensor, offset=drap.offset,
                    ap=[[0, 64]] + list(drap.ap)[1:],
                )
                bcr = np_.tile([64, SB], f32, tag="bcr", name="bcr")
                nc.gpsimd.dma_start(out=bcr, in_=bcast)
                bc = np_.tile([64, SB], f32, tag="bc", name="bc")
                nc.vector.reciprocal(bc, bcr)
                nc.vector.tensor_mul(ao[h][:, s0 : s0 + SB], den[0:64, :], bc)

            def wo_block(wops, sb):
                for sc in range(4 * sb, 4 * sb + 4):
                    c0 = 128 * sc
                    wp = wops.tile([128, HID], f32, tag="ps1", name="wp")
                    nc.tensor.matmul(
                        wp, lhsT=ao[0][:, c0 : c0 + 128], rhs=wo[:, 0:HID],
                        start=True, stop=False,
                    )
                    nc.tensor.matmul(
                        wp, lhsT=ao[1][:, c0 : c0 + 128], rhs=wo[:, HID : 2 * HID],
                        start=False, stop=True,
                    )
                    ob = outp.tile([128, HID], f32, tag="ob", name="ob")
                    nc.vector.tensor_copy(ob, wp)
                    nc.sync.dma_start(out=out_part[c0 : c0 + 128, :], in_=ob)

            with (
                tc.tile_pool(name="qpps", bufs=2, space="PSUM") as qpps,
                tc.tile_pool(name="qkps", bufs=2, space="PSUM") as qkps,
                tc.tile_pool(name="avps", bufs=2, space="PSUM") as avps,
            ):
                # qpps doubles as the vprep transpose pool and the Wo pool:
                # qproj ends as vprep starts, Wo comes later still.
                for sb in range(NSB):
                    s0 = sb * SB
                    qp = qpps.tile([128, SB], f32, tag="ps1", name="qp")
                    for i in range(4):
                        nc.tensor.matmul(
                            qp, lhsT=wq[:, i, :], rhs=xs[i][:, s0 : s0 + SB],
                            start=(i == 0), stop=(i == 3),
                        )
                    nc.vector.tensor_scalar_add(qT[:, s0 : s0 + SB], qp, wqb)
                    vprep(qpps, 0, sb)
                    vprep(qpps, 1, sb)
                    attention(qkps, avps, 0, sb)
                for sb in (3, 2, 1, 0):
                    attention(qkps, avps, 1, sb, tail=(sb <= 1))
                    wo_block(qpps, sb)

    nc.finalize()
    return nc


def _dmask():
    """[128, 2048] mask, segment d in {0..3}: keep (t + 128*d) <= j."""
    t = np.arange(128)[:, None]
    j = np.arange(512)[None, :]
    segs = [(t + 128 * k <= j).astype(np.float32) for k in range(4)]
    return np.concatenate(segs, axis=1)


_NC_CACHE = None


def _get_nc():
    global _NC_CACHE
    if _NC_CACHE is None:
        _NC_CACHE = build_nc()
    return _NC_CACHE


def make_in_maps(x, Wq_w, Wq_b, Wo_w):
    x = np.asarray(x, dtype=np.float32)
    Wq_w = np.asarray(Wq_w, dtype=np.float32)
    Wq_b = np.asarray(Wq_b, dtype=np.float32)
    Wo_w = np.asarray(Wo_w, dtype=np.float32)
    dmask = _dmask()
    in_maps = []
    for c in range(N_CORES):
        b, hp = divmod(c, 4)
        dq = slice(128 * hp, 128 * (hp + 1))
        in_maps.append({
            "xT": np.ascontiguousarray(x[b].T),
            "WqT": np.ascontiguousarray(Wq_w[dq, :].T),
            "Wqb": np.ascontiguousarray(Wq_b[dq].reshape(128, 1)),
            "WoT": np.ascontiguousarray(Wo_w[:, dq].T),
            "dmask": dmask,
        })
    return in_maps


def kernel(x, mask, Wq_w, Wq_b, Wo_w, Wo_b, **_):
    nc = _get_nc()
    in_maps = make_in_maps(x, Wq_w, Wq_b, Wo_w)
    res = run_bass_kernel_spmd(nc, in_maps, core_ids=list(range(N_CORES)))
    Wo_b = np.asarray(Wo_b, dtype=np.float32)
    out = np.empty((B, S, HID), dtype=np.float32)
    for b in range(B):
        acc = res.results[4 * b]["out_part"].astype(np.float32)
        for c in range(4 * b + 1, 4 * b + 4):
            acc = acc + res.results[c]["out_part"]
        out[b] = acc + Wo_b[None, :]
    return out
